# revision 1
# baseline (speedup 1.0000x reference)
"""MLA (multi-head latent attention) Bass kernel for 8 trn2 NeuronCores.

Sharding: core = b*4 + g  (b in {0,1} batches, g in {0..3} head-groups of 4 heads).
Each core computes, for its batch b and 4 heads:
  - projections in feature-major ("transposed") layout from xT (float32r matmuls),
  - flash-style causal attention with scores computed k-major (S^T) so the
    exp'd probabilities feed the PV matmul directly (no transposes),
  - LOBO softmax: attn = exp(s) / (sum_k exp(s) + C*exp(max_k s)); the row max
    is recovered as max_k exp(s) via a DMA max-accumulate (CCE) into a per-head
    comb tile + a DVE 32x32-transpose reduction,
  - row-parallel output projection -> partial [T, E] f32.
Host sums the 4 partials per batch (the all-reduce of the row-parallel design).
"""

import math
import os

import numpy as np

import concourse.bass as bass
import concourse.mybir as mybir
import concourse.tile as _tile_mod
from concourse.tile import TileContext
from concourse.vector_clock import ScopedClock, VectorClock
import bass_rust as _bass_rust
from concourse.bass_utils import run_bass_kernel_spmd

_N_PROCS = _bass_rust.N_PROCS


def _split_drain_and_barrier(self, tick_clock, wait_clock):
    """Replacement for TileContext._drain_and_barrier: the stock version puts
    the whole global vector clock (up to 27 sem waits) on one Drain, which this
    walrus rejects ("Too many sync wait commands").  Emit one Drain per
    outstanding processor instead."""
    gc = tick_clock.global_clock
    procs = [p for p in range(_N_PROCS) if gc[p] > 0]
    for p in procs:
        vc = VectorClock([gc[q] if q == p else 0 for q in range(_N_PROCS)])
        d = self.nc.sync.drain()
        wait_clock.add_sem_waits(d.ins, ScopedClock({None: vc}))
    self.nc.all_engine_barrier()
    popped = self.nc._tile_sem_poison_stack.pop()
    assert popped is self._sem_poison
    self.nc.clear_and_free_semaphores(list(self.sems.allocated().values()))
    self.nc.all_engine_barrier()


_tile_mod.TileContext._drain_and_barrier = _split_drain_and_barrier

# ---------------------------------------------------------------------------
# This walrus build enforces small per-instruction sync-wait budgets
# ("Too many sync wait commands").  Post-process the BIR JSON: any
# instruction carrying more than its budget of waits gets the excess
# hoisted onto same-engine Drain carriers inserted immediately before it
# (same program point on the engine's sequential stream -> semantics
# unchanged).
# ---------------------------------------------------------------------------
_orig_to_json_bytes = bass.Bass.to_json_bytes
_WAIT_LIMITS = {"Drain": 1, "DMACopy": 1}
_DEF_WAIT_LIMIT = 1


def _to_json_split_waits(self, *a, **kw):
    import json as _json
    data = _json.loads(_orig_to_json_bytes(self, *a, **kw))
    nid = 0
    for f in data.get("functions", []):
        for bb in f.get("blocks", []):
            out = []
            for inst in bb.get("instructions", []):
                si = inst.get("sync_info")
                if isinstance(si, dict):
                    w = si.get("on_wait")
                    if isinstance(w, list):
                        k = _WAIT_LIMITS.get(inst.get("opcode"), _DEF_WAIT_LIMIT)
                        if len(w) > k:
                            extra, keep = w[:-k], w[-k:]
                            for wt in extra:
                                out.append({
                                    "debug": inst.get("debug"),
                                    "engine": inst["engine"],
                                    "ins": [], "outs": [],
                                    "name": f"wsplit-{nid}",
                                    "opcode": "Drain",
                                    "sync_info": {"on_update": [],
                                                  "on_wait": [wt]},
                                })
                                nid += 1
                            si["on_wait"] = keep
                out.append(inst)
            bb["instructions"] = out
    return _json.dumps(data).encode()


bass.Bass.to_json_bytes = _to_json_split_waits

B, T, E = 2, 2048, 1024
H, DH = 16, 64
DKV = 256
DR = 32
HL = 4              # heads per core
NG = 4              # head groups
SCALE = 1.0 / math.sqrt(DH + DR)
TG = 512            # query-group width
KC = 128            # key-chunk width
NTG = T // TG       # 4
NKC = T // KC       # 16
EC = E // 128       # 8  e-chunks
CC = DKV // 128     # 2  latent chunks

F32 = mybir.dt.float32
F32R = mybir.dt.float32r
BF16 = mybir.dt.bfloat16
AF = mybir.ActivationFunctionType
ALU = mybir.AluOpType
AX = mybir.AxisListType

_CACHE = {}


def _r(ap):
    return ap.bitcast(F32R)


def _build_program():
    nc = bass.Bass()

    xT = nc.declare_dram_parameter("xT", [E, T], F32, isOutput=False)
    wq = nc.declare_dram_parameter("wq", [E, HL * DH], F32, isOutput=False)
    wqr = nc.declare_dram_parameter("wqr", [E, HL * DR], F32, isOutput=False)
    wkr = nc.declare_dram_parameter("wkr", [E, DR], F32, isOutput=False)
    wkvd = nc.declare_dram_parameter("wkvd", [E, DKV], F32, isOutput=False)
    wku = nc.declare_dram_parameter("wku", [DKV, HL * DH], F32, isOutput=False)
    wvu = nc.declare_dram_parameter("wvu", [DKV, HL * DH], F32, isOutput=False)
    wo = nc.declare_dram_parameter("wo", [HL * DH, E], F32, isOutput=False)
    cosq = nc.declare_dram_parameter("cosq", [HL * DR, T], F32, isOutput=False)
    sinq = nc.declare_dram_parameter("sinq", [HL * DR, T], F32, isOutput=False)
    lobo = nc.declare_dram_parameter("lobo", [HL, 1], F32, isOutput=False)
    masks = nc.declare_dram_parameter("masks", [128, 4 * TG], F32, isOutput=False)
    out = nc.declare_dram_parameter("out", [T, E], F32, isOutput=True)

    with TileContext(nc) as tc:
        from contextlib import ExitStack

        with ExitStack() as ctx:
            singles = ctx.enter_context(tc.tile_pool(name="singles", bufs=1))
            pool = ctx.enter_context(tc.tile_pool(name="pool", bufs=2))
            psp = ctx.enter_context(tc.tile_pool(name="psp", bufs=1, space="PSUM"))

            # ---------------- weights (f32; x-side used as f32r) ----------------
            wq_sb = singles.tile([128, EC, HL * DH], BF16)
            nc.gpsimd.dma_start(
                out=wq_sb, in_=wq.rearrange("(c p) f -> p c f", p=128))
            wqr_sb = singles.tile([128, EC, HL * DR], BF16)
            nc.gpsimd.dma_start(
                out=wqr_sb, in_=wqr.rearrange("(c p) f -> p c f", p=128))
            wkr_sb = singles.tile([128, EC, DR], BF16)
            nc.gpsimd.dma_start(
                out=wkr_sb, in_=wkr.rearrange("(c p) f -> p c f", p=128))
            wkvd_sb = singles.tile([128, EC, DKV], BF16)
            nc.gpsimd.dma_start(
                out=wkvd_sb, in_=wkvd.rearrange("(c p) f -> p c f", p=128))
            # latent-side weights in bf16 (latT is bf16)
            wku_sb = singles.tile([128, CC, HL * DH], BF16)
            nc.gpsimd.dma_start(
                out=wku_sb, in_=wku.rearrange("(c p) f -> p c f", p=128))
            wvu_sb = singles.tile([128, CC, HL * DH], BF16)
            nc.gpsimd.dma_start(
                out=wvu_sb, in_=wvu.rearrange("(c p) f -> p c f", p=128))
            wo_sb = singles.tile([128, 2, E], BF16)
            nc.gpsimd.dma_start(
                out=wo_sb, in_=wo.rearrange("(c p) e -> p c e", p=128))

            cosq_sb = singles.tile([128, T], BF16)
            nc.gpsimd.dma_start(out=cosq_sb, in_=cosq[:, :])
            sinq_sb = singles.tile([128, T], BF16)
            nc.gpsimd.dma_start(out=sinq_sb, in_=sinq[:, :])
            lobo_sb = singles.tile([HL, 1], F32)
            nc.sync.dma_start(out=lobo_sb, in_=lobo[:, :])
            c_sb = singles.tile([HL, 1], F32)
            nc.scalar.activation(c_sb, lobo_sb, AF.Exp)

            # causal masks for the 4 diagonal offsets: keep iff x - y - 128*j <= 0
            masks_sb = singles.tile([128, 4, TG], BF16)
            nc.gpsimd.dma_start(
                out=masks_sb, in_=masks.rearrange("p (j y) -> p j y", j=4))

            ones_sb = singles.tile([1, DH], F32)
            nc.vector.memset(ones_sb, 1.0)

            # ---------------- persistent activation tiles ----------------
            latT_sb = singles.tile([128, CC, T], BF16)
            qT = [singles.tile([96, T], BF16, name=f"qT{h}") for h in range(HL)]
            kT = [singles.tile([96, T], BF16, name=f"kT{h}") for h in range(HL)]
            rp_pre = singles.tile([128, T], BF16)
            rp_swap = singles.tile([128, T], BF16)
            rp_m1 = singles.tile([128, T], BF16)
            rp_m2 = singles.tile([128, T], BF16)
            kr_pre = singles.tile([DR, T], BF16)
            xt_sb = singles.tile([128, EC, T], BF16)
            nc.gpsimd.dma_start(
                out=xt_sb, in_=xT.rearrange("(c p) t -> p c t", p=128))

            # ---------------- projections from xT, streamed per tg ----------------
            for tg in range(NTG):
                ts = slice(tg * TG, (tg + 1) * TG)
                xts = [xt_sb[:, ec, ts] for ec in range(EC)]
                # latent halves + k_rope
                pa = psp.tile([128, TG], F32, name="pa", tag="A", bufs=3)
                pb = psp.tile([128, TG], F32, name="pb", tag="B", bufs=3)
                pc = psp.tile([128, TG], F32, name="pc", tag="C", bufs=2)
                for ec in range(EC):
                    nc.tensor.matmul(
                        pa, (wkvd_sb[:, ec, 0:128]), (xts[ec]),
                        start=(ec == 0), stop=(ec == EC - 1))
                    nc.tensor.matmul(
                        pb, (wkvd_sb[:, ec, 128:256]), (xts[ec]),
                        start=(ec == 0), stop=(ec == EC - 1))
                    nc.tensor.matmul(
                        pc[0:DR, :], (wkr_sb[:, ec, :]), (xts[ec]),
                        start=(ec == 0), stop=(ec == EC - 1))
                nc.vector.tensor_copy(latT_sb[:, 0, ts], pa)
                nc.vector.tensor_copy(latT_sb[:, 1, ts], pb)
                nc.scalar.copy(kr_pre[:, ts], pc[0:DR, :])
                # q projections
                pa = psp.tile([128, TG], F32, name="pa", tag="A", bufs=3)
                pb = psp.tile([128, TG], F32, name="pb", tag="B", bufs=3)
                pc = psp.tile([128, TG], F32, name="pc", tag="C", bufs=2)
                for ec in range(EC):
                    nc.tensor.matmul(
                        pa, (wq_sb[:, ec, 0:128]), (xts[ec]),
                        start=(ec == 0), stop=(ec == EC - 1))
                    nc.tensor.matmul(
                        pb, (wq_sb[:, ec, 128:256]), (xts[ec]),
                        start=(ec == 0), stop=(ec == EC - 1))
                    nc.tensor.matmul(
                        pc, (wqr_sb[:, ec, :]), (xts[ec]),
                        start=(ec == 0), stop=(ec == EC - 1))
                st = pool.tile([128, TG], BF16, name="st0", tag="qkstage", bufs=3)
                nc.scalar.copy(st, pa)
                nc.sync.dma_start(out=qT[0][0:DH, ts], in_=st[0:DH, :])
                nc.sync.dma_start(out=qT[1][0:DH, ts], in_=st[DH:128, :])
                st = pool.tile([128, TG], BF16, name="st1", tag="qkstage", bufs=3)
                nc.scalar.copy(st, pb)
                nc.sync.dma_start(out=qT[2][0:DH, ts], in_=st[0:DH, :])
                nc.sync.dma_start(out=qT[3][0:DH, ts], in_=st[DH:128, :])
                nc.scalar.copy(rp_pre[:, ts], pc)
                # rope on q_r rows for this tg
                for h in range(HL):
                    nc.sync.dma_start(
                        out=rp_swap[h * DR:h * DR + 16, ts],
                        in_=rp_pre[h * DR + 16:h * DR + 32, ts])
                    nc.sync.dma_start(
                        out=rp_swap[h * DR + 16:h * DR + 32, ts],
                        in_=rp_pre[h * DR:h * DR + 16, ts])
                nc.vector.tensor_mul(rp_m1[:, ts], rp_pre[:, ts], cosq_sb[:, ts])
                nc.vector.tensor_mul(rp_m2[:, ts], rp_swap[:, ts], sinq_sb[:, ts])
                nc.vector.tensor_add(rp_m2[:, ts], rp_m1[:, ts], rp_m2[:, ts])
                for h in range(HL):
                    nc.sync.dma_start(
                        out=qT[h][DH:96, ts], in_=rp_m2[h * DR:(h + 1) * DR, ts])
                # rope on k_r rows for this tg
                nc.sync.dma_start(
                    out=rp_swap[0:16, ts], in_=kr_pre[16:32, ts])
                nc.sync.dma_start(
                    out=rp_swap[16:32, ts], in_=kr_pre[0:16, ts])
                nc.vector.tensor_mul(
                    rp_m1[0:DR, ts], kr_pre[:, ts], cosq_sb[0:DR, ts])
                nc.vector.tensor_mul(
                    rp_m2[0:DR, ts], rp_swap[0:DR, ts], sinq_sb[0:DR, ts])
                nc.vector.tensor_add(
                    rp_m2[0:DR, ts], rp_m1[0:DR, ts], rp_m2[0:DR, ts])
                for h in range(HL):
                    nc.sync.dma_start(out=kT[h][DH:96, ts], in_=rp_m2[0:DR, ts])

            # ---------------- k_c from latentT ----------------
            for hp in range(2):
                for tg in range(NTG):
                    ts = slice(tg * TG, (tg + 1) * TG)
                    pa = psp.tile([128, TG], F32, name="pa", tag="A", bufs=3)
                    for cc in range(CC):
                        nc.tensor.matmul(
                            pa, wku_sb[:, cc, hp * 128:(hp + 1) * 128],
                            latT_sb[:, cc, ts],
                            start=(cc == 0), stop=(cc == CC - 1))
                    st = pool.tile([128, TG], BF16, name="st2", tag="qkstage", bufs=3)
                    nc.vector.tensor_copy(st, pa)
                    nc.sync.dma_start(out=kT[2 * hp][0:DH, ts], in_=st[0:DH, :])
                    nc.sync.dma_start(
                        out=kT[2 * hp + 1][0:DH, ts], in_=st[DH:128, :])

            # ---------------- V (natural layout, +ones column) ----------------
            v_sb = singles.tile([128, NKC, HL, DH + 1], BF16)
            nc.vector.memset(v_sb, 1.0)
            for tt in range(NKC):
                pb = psp.tile([128, HL * DH], F32, name="pv", tag="B", bufs=3)
                for cc in range(CC):
                    nc.tensor.matmul(
                        pb, latT_sb[:, cc, tt * 128:(tt + 1) * 128],
                        wvu_sb[:, cc, :],
                        start=(cc == 0), stop=(cc == CC - 1))
                nc.vector.tensor_copy(v_sb[:, tt, :, 0:DH], pb)

            # ---------------- attention ----------------
            yraw_sb = singles.tile([DH, HL, T], BF16)
            dsum_sb = singles.tile([HL, T], F32)
            emax_sb = singles.tile([HL, T], F32)
            emst_sb = singles.tile([HL, T], F32)

            for h in range(HL):
                comb = pool.tile([128, T], BF16, name="comb", tag="comb", bufs=1)
                nc.vector.memset(comb, 0.0)
                for qg in range(NTG):
                    qs = slice(qg * TG, (qg + 1) * TG)
                    nkc = 4 * qg + 4
                    yps = psp.tile([DH + 1, TG], F32, name="py", tag="B", bufs=3)
                    for kc in range(nkc):
                        sps = psp.tile([128, TG], F32, name="ps", tag="A", bufs=3)
                        nc.tensor.matmul(
                            sps, kT[h][:, kc * KC:(kc + 1) * KC], qT[h][:, qs])
                        pt = pool.tile(
                            [128, TG], BF16, name="pt", tag="ptile", bufs=4)
                        nc.scalar.activation(pt, sps, AF.Exp, scale=SCALE)
                        j = kc - 4 * qg
                        if j >= 0:
                            nc.gpsimd.tensor_mul(pt, pt, masks_sb[:, j, :])
                        nc.vector.tensor_max(comb[:, qs], comb[:, qs], pt)
                        nc.tensor.matmul(
                            yps, v_sb[:, kc, h, :], pt,
                            start=(kc == 0), stop=(kc == nkc - 1))
                    nc.scalar.copy(yraw_sb[:, h, qs], yps[0:DH, :])
                    std = pool.tile([DH + 1, TG], F32, name="std", tag="stgd", bufs=1)
                    nc.scalar.copy(std[DH:DH + 1, :], yps[DH:DH + 1, :])
                    nc.sync.dma_start(
                        out=dsum_sb[h:h + 1, qs], in_=std[DH:DH + 1, :])
                # emax for this head: partition-max of comb via 32x32 transpose
                combT = pool.tile([128, T], BF16, name="combT", tag="combT", bufs=1)
                nc.vector.transpose(combT, comb)
                red = pool.tile([128, T // 32], F32, name="red", tag="red", bufs=1)
                nc.vector.reduce_max(
                    red, combT.rearrange("p (b j) -> p b j", j=32), axis=AX.X)
                stk = pool.tile([32, 4, T // 32], F32, name="stk", tag="stk", bufs=1)
                for a in range(4):
                    nc.sync.dma_start(
                        out=stk[:, a, :], in_=red[a * 32:(a + 1) * 32, :])
                emf = pool.tile([32, T // 32], F32, name="emf", tag="emf", bufs=1)
                nc.vector.reduce_max(
                    emf, stk.rearrange("p a b -> p b a"), axis=AX.X)
                nc.sync.dma_start(out=emst_sb[h:h + 1, :], in_=emf)

            # ---------------- denominators + normalize ----------------
            # un-permute the per-head maxes (i-major -> natural q order)
            nc.vector.tensor_copy(
                emax_sb.rearrange("p (b i) -> p i b", i=32),
                emst_sb.rearrange("p (i b) -> p i b", b=64))
            # d = dsum + C * emax  (in place into dsum), r = 1/d (into emax)
            nc.vector.scalar_tensor_tensor(
                out=dsum_sb, in0=emax_sb, scalar=c_sb, in1=dsum_sb,
                op0=ALU.mult, op1=ALU.add)
            nc.vector.reciprocal(emax_sb, dsum_sb)

            yT_sb = singles.tile([128, 2, T], BF16)
            for h in range(HL):
                for qg in range(NTG):
                    qs = slice(qg * TG, (qg + 1) * TG)
                    rhh = pool.tile([1, TG], F32, name="rh", tag="rh", bufs=2)
                    nc.sync.dma_start(out=rhh, in_=emax_sb[h:h + 1, qs])
                    bc = psp.tile([DH, TG], F32, name="bc", tag="C", bufs=2)
                    nc.tensor.matmul(bc, ones_sb, rhh)
                    yn = pool.tile([DH, TG], BF16, name="yn", tag="yn", bufs=3)
                    nc.vector.tensor_mul(yn, yraw_sb[:, h, qs], bc)
                    nc.sync.dma_start(
                        out=yT_sb[(h % 2) * DH:(h % 2 + 1) * DH, h // 2, qs],
                        in_=yn)

            # ---------------- output projection (row-parallel partial) ----------------
            for tt in range(NKC):
                for eg in range(2):
                    pa = psp.tile([128, TG], F32, name="po", tag="A", bufs=3)
                    for fc in range(2):
                        nc.tensor.matmul(
                            pa, yT_sb[:, fc, tt * 128:(tt + 1) * 128],
                            wo_sb[:, fc, eg * TG:(eg + 1) * TG],
                            start=(fc == 0), stop=(fc == 1))
                    ost = pool.tile([128, TG], F32, name="ost", tag="ost", bufs=2)
                    if (tt + eg) % 2 == 0:
                        nc.scalar.copy(ost, pa)
                    else:
                        nc.vector.tensor_copy(ost, pa)
                    nc.sync.dma_start(
                        out=out[tt * 128:(tt + 1) * 128, eg * TG:(eg + 1) * TG],
                        in_=ost)

    return nc


def _masks():
    x = np.arange(128)[:, None]
    y = np.arange(TG)[None, :]
    ms = [(x - y + 128 * j <= 0).astype(np.float32) for j in range(4)]
    return np.concatenate(ms, axis=1)  # [128, 4*TG]


def _rope_tables():
    half = DR // 2
    inv = 1.0 / (10000.0 ** (np.arange(half, dtype=np.float64) / half))
    ang = np.arange(T, dtype=np.float64)[:, None] * inv[None, :]  # (T, half)
    cos = np.cos(ang).T  # (half, T)
    sin = np.sin(ang).T
    cosk = np.concatenate([cos, cos], axis=0)                 # (32, T)
    sink = np.concatenate([-sin, sin], axis=0)
    cosq = np.tile(cosk, (HL, 1)).astype(np.float32)          # (128, T)
    sinq = np.tile(sink, (HL, 1)).astype(np.float32)
    return cosq, sinq


def kernel(x, Wq, Wqr, Wkr, Wkvd, Wku, Wvu, Wo, lobo_log):
    x = np.asarray(x, dtype=np.float32)
    Wq = np.asarray(Wq, dtype=np.float32)
    Wqr = np.asarray(Wqr, dtype=np.float32)
    Wkr = np.asarray(Wkr, dtype=np.float32)
    Wkvd = np.asarray(Wkvd, dtype=np.float32)
    Wku = np.asarray(Wku, dtype=np.float32)
    Wvu = np.asarray(Wvu, dtype=np.float32)
    Wo = np.asarray(Wo, dtype=np.float32)
    lobo_log = np.asarray(lobo_log, dtype=np.float32)

    if "nc" not in _CACHE:
        _CACHE["nc"] = _build_program()
    nc = _CACHE["nc"]

    cosq, sinq = _rope_tables()
    msk = _masks()
    in_maps = []
    for core in range(8):
        b, g = core // NG, core % NG
        hs = slice(g * HL * DH, (g + 1) * HL * DH)
        rs = slice(g * HL * DR, (g + 1) * HL * DR)
        in_maps.append({
            "xT": np.ascontiguousarray(x[b].T),
            "wq": np.ascontiguousarray(Wq[:, hs]),
            "wqr": np.ascontiguousarray(Wqr[:, rs]),
            "wkr": Wkr,
            "wkvd": Wkvd,
            "wku": np.ascontiguousarray(Wku[:, hs]),
            "wvu": np.ascontiguousarray(Wvu[:, hs]),
            "wo": np.ascontiguousarray(Wo[hs, :]),
            "cosq": cosq, "sinq": sinq, "masks": msk,
            "lobo": np.ascontiguousarray(
                lobo_log[g * HL:(g + 1) * HL].reshape(HL, 1)),
        })

    trace = bool(os.environ.get("BASS_TRACE_KERNEL"))
    bkr = run_bass_kernel_spmd(
        nc, in_maps, core_ids=list(range(8)), trace=trace)
    if trace:
        print(f"HW exec time: {bkr.exec_time_ns} ns")
        if bkr.instructions_and_trace is not None:
            print("trace:", bkr.instructions_and_trace[1])
        _CACHE["last_result"] = bkr
    res = bkr.results
    out = np.zeros((B, T, E), dtype=np.float32)
    for core in range(8):
        out[core // NG] += res[core]["out"]
    return out



# revision 11
# speedup vs baseline: 1.1217x; 1.1217x over previous
"""MLA (multi-head latent attention) Bass kernel for 8 trn2 NeuronCores.

Sharding: core = b*4 + g  (b in {0,1} batches, g in {0..3} head-groups of 4
heads).  Each core computes, for its batch b and 4 heads, a row-parallel
partial of the output projection; the host sums the 4 partials per batch.

Design (v2):
 - All inputs pre-converted to bf16 on the host (halves input DMA, enables
   HW-DGE queues).
 - Projections stream per 512-col tile group (tg); attention runs qg-outer
   (query groups of 512), head pairs inner, flash-style over 128-key chunks
   with column-trimmed diagonal groups (block-causal at 128 granularity).
 - Scores = two accumulating matmuls per head (k_c x q_c with K=64, rope
   K=32), row-tiled so head pairs run concurrently in the PE array.  No
   per-head q/k assembly copies at all.
 - exp on ScalarE (PSUM->SBUF bf16), triangle-only causal masks ([128,128]
   per diagonal chunk) on DVE, per-chunk running max split DVE/GpSimd.
 - LOBO denominator: d = sum_k e + C * max_k e.  sum via a ones-column in
   the V stationary operand; max via comb folds (128->16) + one PE
   transpose; both land transposed ([q%128, block]) so a single batched
   fast-reciprocal serves a whole query group.
 - Per-head normalization via a K=1 broadcast matmul of 1/d rows, Wo
   row-parallel with bf16 output (host sums partials in f32).
 - Projection / Wo matmuls are dripped into the attention stream as PE
   filler so the tensor engine never idles (keeps HAM un-throttled).
"""

import math
import os
from collections import deque

import numpy as np

import concourse.bass as bass
import concourse.mybir as mybir
import concourse.tile as _tile_mod
from concourse.tile import TileContext
from concourse.vector_clock import ScopedClock, VectorClock
import bass_rust as _bass_rust
from concourse.bass_utils import run_bass_kernel_spmd

_N_PROCS = _bass_rust.N_PROCS


def _split_drain_and_barrier(self, tick_clock, wait_clock):
    """Replacement for TileContext._drain_and_barrier: the stock version puts
    the whole global vector clock (up to 27 sem waits) on one Drain, which this
    walrus rejects ("Too many sync wait commands").  Emit one Drain per
    outstanding processor instead."""
    gc = tick_clock.global_clock
    procs = [p for p in range(_N_PROCS) if gc[p] > 0]
    for p in procs:
        vc = VectorClock([gc[q] if q == p else 0 for q in range(_N_PROCS)])
        d = self.nc.sync.drain()
        wait_clock.add_sem_waits(d.ins, ScopedClock({None: vc}))
    self.nc.all_engine_barrier()
    popped = self.nc._tile_sem_poison_stack.pop()
    assert popped is self._sem_poison
    self.nc.clear_and_free_semaphores(list(self.sems.allocated().values()))
    self.nc.all_engine_barrier()


_tile_mod.TileContext._drain_and_barrier = _split_drain_and_barrier

# ---------------------------------------------------------------------------
# This walrus build enforces small per-instruction sync-wait budgets
# ("Too many sync wait commands").  Post-process the BIR JSON: any
# instruction carrying more than its budget of waits gets the excess
# hoisted onto same-engine Drain carriers inserted immediately before it
# (same program point on the engine's sequential stream -> semantics
# unchanged).
# ---------------------------------------------------------------------------
_orig_to_json_bytes = bass.Bass.to_json_bytes
_WAIT_LIMITS = {"Drain": 1, "DMACopy": 1}
_DEF_WAIT_LIMIT = 1


def _to_json_split_waits(self, *a, **kw):
    import json as _json
    data = _json.loads(_orig_to_json_bytes(self, *a, **kw))
    nid = 0
    for f in data.get("functions", []):
        for bb in f.get("blocks", []):
            out = []
            for inst in bb.get("instructions", []):
                si = inst.get("sync_info")
                if isinstance(si, dict):
                    w = si.get("on_wait")
                    if isinstance(w, list):
                        k = _WAIT_LIMITS.get(inst.get("opcode"), _DEF_WAIT_LIMIT)
                        if len(w) > k:
                            extra, keep = w[:-k], w[-k:]
                            for wt in extra:
                                out.append({
                                    "debug": inst.get("debug"),
                                    "engine": inst["engine"],
                                    "ins": [], "outs": [],
                                    "name": f"wsplit-{nid}",
                                    "opcode": "Drain",
                                    "sync_info": {"on_update": [],
                                                  "on_wait": [wt]},
                                })
                                nid += 1
                            si["on_wait"] = keep
                out.append(inst)
            bb["instructions"] = out
    return _json.dumps(data).encode()


bass.Bass.to_json_bytes = _to_json_split_waits

B, T, E = 2, 2048, 1024
H, DH = 16, 64
DKV = 256
DR = 32
HL = 4              # heads per core
NG = 4              # head groups
SCALE = 1.0 / math.sqrt(DH + DR)
TG = 512            # query-group width
KC = 128            # key-chunk width
NTG = T // TG       # 4
NKC = T // KC       # 16
EC = E // 128       # 8  e-chunks
CC = DKV // 128     # 2  latent chunks
VW = 68             # per-head stride in v_sb (64 data + ones col + pad)

F32 = mybir.dt.float32
BF16 = mybir.dt.bfloat16
AF = mybir.ActivationFunctionType
ALU = mybir.AluOpType
AX = mybir.AxisListType

_CACHE = {}


def _build_program():
    nc = bass.Bass()

    xT = nc.declare_dram_parameter("xT", [E, T], BF16, isOutput=False)
    wq = nc.declare_dram_parameter("wq", [E, HL * DH], BF16, isOutput=False)
    wqr = nc.declare_dram_parameter("wqr", [E, HL * DR], BF16, isOutput=False)
    wkr = nc.declare_dram_parameter("wkr", [E, DR], BF16, isOutput=False)
    wkvd = nc.declare_dram_parameter("wkvd", [E, DKV], BF16, isOutput=False)
    wku = nc.declare_dram_parameter("wku", [DKV, HL * DH], BF16, isOutput=False)
    wvu = nc.declare_dram_parameter("wvu", [DKV, HL * DH], BF16, isOutput=False)
    wo = nc.declare_dram_parameter("wo", [HL * DH, E], BF16, isOutput=False)
    cosq = nc.declare_dram_parameter("cosq", [HL * DR, T], BF16, isOutput=False)
    sinq = nc.declare_dram_parameter("sinq", [HL * DR, T], BF16, isOutput=False)
    tri = nc.declare_dram_parameter("tri", [128, 128], BF16, isOutput=False)
    eyeb = nc.declare_dram_parameter("eyeb", [128, 128], BF16, isOutput=False)
    eyef = nc.declare_dram_parameter("eyef", [128, 128], F32, isOutput=False)
    lobo = nc.declare_dram_parameter("lobo", [1, HL], F32, isOutput=False)
    out = nc.declare_dram_parameter("out", [T, E], BF16, isOutput=True)

    with TileContext(nc) as tc:
        from contextlib import ExitStack

        with ExitStack() as ctx:
            singles = ctx.enter_context(tc.tile_pool(name="singles", bufs=1))
            pool = ctx.enter_context(tc.tile_pool(name="pool", bufs=2))
            psp = ctx.enter_context(tc.tile_pool(name="psp", bufs=1, space="PSUM"))

            # ---------------- initial loads (HW DGE queues; all bf16) -------
            xt_sb = singles.tile([128, EC, T], BF16)
            xr = xT.rearrange("(c p) t -> p c t", p=128)
            # tg0 columns first so projections can start early
            nc.sync.dma_start(out=xt_sb[:, :, 0:TG], in_=xr[:, :, 0:TG])

            wkvd_sb = singles.tile([128, EC, DKV], BF16)
            nc.sync.dma_start(
                out=wkvd_sb, in_=wkvd.rearrange("(c p) f -> p c f", p=128))
            wq_sb = singles.tile([128, EC, HL * DH], BF16)
            nc.sync.dma_start(
                out=wq_sb, in_=wq.rearrange("(c p) f -> p c f", p=128))
            wqr_sb = singles.tile([128, EC, HL * DR], BF16)
            nc.sync.dma_start(
                out=wqr_sb, in_=wqr.rearrange("(c p) f -> p c f", p=128))
            wkr_sb = singles.tile([128, EC, DR], BF16)
            nc.sync.dma_start(
                out=wkr_sb, in_=wkr.rearrange("(c p) f -> p c f", p=128))
            cosq_sb = singles.tile([128, T], BF16)
            nc.scalar.dma_start(out=cosq_sb, in_=cosq[:, :])
            sinq_sb = singles.tile([128, T], BF16)
            nc.scalar.dma_start(out=sinq_sb, in_=sinq[:, :])
            tri_sb = singles.tile([128, 128], BF16)
            nc.scalar.dma_start(out=tri_sb, in_=tri[:, :])
            eyeb_sb = singles.tile([128, 128], BF16)
            nc.scalar.dma_start(out=eyeb_sb, in_=eyeb[:, :])
            eyef_sb = singles.tile([128, 128], F32)
            nc.scalar.dma_start(out=eyef_sb, in_=eyef[:, :])
            lobo_sb = singles.tile([1, HL], F32)
            nc.scalar.dma_start(out=lobo_sb, in_=lobo[:, :])
            wku_sb = singles.tile([128, CC, HL * DH], BF16)
            nc.scalar.dma_start(
                out=wku_sb, in_=wku.rearrange("(c p) f -> p c f", p=128))
            wvu_sb = singles.tile([128, CC, HL * DH], BF16)
            nc.scalar.dma_start(
                out=wvu_sb, in_=wvu.rearrange("(c p) f -> p c f", p=128))
            # rest of x / wo while tg0 projections run
            for tg in range(1, NTG):
                ts = slice(tg * TG, (tg + 1) * TG)
                eng = nc.scalar if tg == 2 else nc.sync
                eng.dma_start(out=xt_sb[:, :, ts], in_=xr[:, :, ts])
            wo_sb = singles.tile([128, 2, E], BF16)
            nc.sync.dma_start(
                out=wo_sb, in_=wo.rearrange("(c p) e -> p c e", p=128))

            # ---------------- small constants -------------------------------
            # C = exp(lobo) as a row, broadcast to all 128 partitions
            c_row = singles.tile([1, HL], F32)
            nc.scalar.activation(c_row, lobo_sb, AF.Exp)  # also warms exp table
            ones1 = singles.tile([1, 128], BF16)
            nc.vector.memset(ones1, 1.0)
            ones1f = singles.tile([1, 128], F32)
            nc.vector.memset(ones1f, 1.0)
            cb_ps = psp.tile([128, HL], F32, name="cb_ps", tag="pj", bufs=3)
            nc.tensor.matmul(cb_ps, ones1f, c_row, start=True, stop=True)
            c_bcast = singles.tile([128, HL], F32)
            nc.vector.tensor_copy(c_bcast, cb_ps)

            # ---------------- persistent activations ------------------------
            qc2 = [singles.tile([128, T], BF16, name=f"qc2_{p}") for p in range(2)]
            qr_t = singles.tile([128, T], BF16)
            kc2 = [singles.tile([128, T], BF16, name=f"kc2_{p}") for p in range(2)]
            krT = singles.tile([128, T], BF16)      # kr rope replicated 4x
            latT = singles.tile([128, CC, T], BF16)
            v_sb = singles.tile([128, NKC, HL, VW], BF16)
            nc.vector.memset(v_sb, 1.0)             # ones col (64) + pad
            yns = [singles.tile([128, T], BF16, name=f"yns_{p}") for p in range(2)]

            # rope scratch (full-T so tg slices are independent)
            rp_pre = singles.tile([128, T], BF16)
            rp_swap = singles.tile([128, T], BF16)
            rp_m1 = singles.tile([128, T], BF16)
            kr_pre = singles.tile([DR, T], BF16)
            kr_swap = singles.tile([DR, T], BF16)
            kr_m1 = singles.tile([DR, T], BF16)

            # transpose staging (rows 68-127 stay zero forever)
            tp_in = singles.tile([128, 128], BF16)
            nc.vector.memset(tp_in, 0.0)

            dAll = singles.tile([128, NTG, HL, 4], F32)
            rAll = singles.tile([128, NTG, HL, 4], F32)

            # ---------------- projection / Wo units (PE filler) -------------
            proj_fill = deque()
            wo_fill = deque()

            def drip(q, n):
                for _ in range(n):
                    if not q:
                        return
                    q.popleft()()

            def flushq(q):
                while q:
                    q.popleft()()

            def make_proj_units(tg):
                ts = slice(tg * TG, (tg + 1) * TG)
                xts = [xt_sb[:, ec, ts] for ec in range(EC)]
                units = []

                def u_kvd(cc):
                    def f():
                        pa = psp.tile([128, TG], F32, name="pa", tag="pj", bufs=3)
                        for ec in range(EC):
                            nc.tensor.matmul(
                                pa, wkvd_sb[:, ec, cc * 128:(cc + 1) * 128],
                                xts[ec], start=(ec == 0), stop=(ec == EC - 1))
                        nc.vector.tensor_copy(latT[:, cc, ts], pa)
                    return f

                def u_kr():
                    def f():
                        pc = psp.tile([32, TG], F32, name="pc", tag="pj", bufs=3)
                        for ec in range(EC):
                            nc.tensor.matmul(
                                pc, wkr_sb[:, ec, :], xts[ec],
                                start=(ec == 0), stop=(ec == EC - 1))
                        nc.vector.tensor_copy(kr_pre[:, ts], pc)
                        nc.sync.dma_start(
                            out=kr_swap[0:16, ts], in_=kr_pre[16:32, ts])
                        nc.sync.dma_start(
                            out=kr_swap[16:32, ts], in_=kr_pre[0:16, ts])
                        nc.gpsimd.tensor_mul(
                            kr_m1[:, ts], kr_pre[:, ts], cosq_sb[0:DR, ts])
                        nc.gpsimd.tensor_mul(
                            kr_swap[:, ts], kr_swap[:, ts], sinq_sb[0:DR, ts])
                        nc.gpsimd.tensor_add(
                            krT[0:DR, ts], kr_m1[:, ts], kr_swap[:, ts])
                        for i in range(1, 4):
                            nc.sync.dma_start(
                                out=krT[i * DR:(i + 1) * DR, ts],
                                in_=krT[0:DR, ts])
                    return f

                def u_qc(p):
                    def f():
                        pa = psp.tile([128, TG], F32, name="pq", tag="pj", bufs=3)
                        for ec in range(EC):
                            nc.tensor.matmul(
                                pa, wq_sb[:, ec, p * 128:(p + 1) * 128],
                                xts[ec], start=(ec == 0), stop=(ec == EC - 1))
                        nc.vector.tensor_copy(qc2[p][:, ts], pa)
                    return f

                def u_qr():
                    def f():
                        pa = psp.tile([128, TG], F32, name="pr", tag="pj", bufs=3)
                        for ec in range(EC):
                            nc.tensor.matmul(
                                pa, wqr_sb[:, ec, :], xts[ec],
                                start=(ec == 0), stop=(ec == EC - 1))
                        nc.vector.tensor_copy(rp_pre[:, ts], pa)
                        for h in range(HL):
                            nc.sync.dma_start(
                                out=rp_swap[h * DR:h * DR + 16, ts],
                                in_=rp_pre[h * DR + 16:h * DR + 32, ts])
                            nc.sync.dma_start(
                                out=rp_swap[h * DR + 16:h * DR + 32, ts],
                                in_=rp_pre[h * DR:h * DR + 16, ts])
                        nc.gpsimd.tensor_mul(
                            rp_m1[:, ts], rp_pre[:, ts], cosq_sb[:, ts])
                        nc.gpsimd.tensor_mul(
                            rp_swap[:, ts], rp_swap[:, ts], sinq_sb[:, ts])
                        nc.gpsimd.tensor_add(
                            qr_t[:, ts], rp_m1[:, ts], rp_swap[:, ts])
                    return f

                def u_kc(p):
                    def f():
                        pa = psp.tile([128, TG], F32, name="pk", tag="pj", bufs=3)
                        for cc in range(CC):
                            nc.tensor.matmul(
                                pa, wku_sb[:, cc, p * 128:(p + 1) * 128],
                                latT[:, cc, ts],
                                start=(cc == 0), stop=(cc == CC - 1))
                        nc.vector.tensor_copy(kc2[p][:, ts], pa)
                    return f

                def u_v(tt):
                    def f():
                        pb = psp.tile([128, HL * DH], F32, name="pv", tag="pj",
                                      bufs=3)
                        for cc in range(CC):
                            nc.tensor.matmul(
                                pb, latT[:, cc, tt * 128:(tt + 1) * 128],
                                wvu_sb[:, cc, :],
                                start=(cc == 0), stop=(cc == CC - 1))
                        nc.vector.tensor_copy(
                            v_sb[:, tt, :, 0:DH],
                            pb.rearrange("p (h d) -> p h d", h=HL))
                    return f

                units.append(u_kvd(0))
                units.append(u_kvd(1))
                units.append(u_kr())
                units.append(u_qc(0))
                units.append(u_qc(1))
                units.append(u_qr())
                units.append(u_kc(0))
                units.append(u_kc(1))
                for tt in range(4 * tg, 4 * tg + 4):
                    units.append(u_v(tt))
                return units

            def make_wo_units(qg):
                units = []

                def u_wo(tt, eg):
                    def f():
                        po = psp.tile([128, TG], F32, name="po", tag="pj", bufs=3)
                        for p in range(2):
                            nc.tensor.matmul(
                                po, yns[p][:, tt * 128:(tt + 1) * 128],
                                wo_sb[:, p, eg * TG:(eg + 1) * TG],
                                start=(p == 0), stop=(p == 1))
                        ob = pool.tile([128, TG], BF16, name="ob", tag="ob",
                                       bufs=3)
                        nc.vector.tensor_copy(ob, po)
                        nc.sync.dma_start(
                            out=out[tt * 128:(tt + 1) * 128,
                                    eg * TG:(eg + 1) * TG],
                            in_=ob)
                    return f

                for tt in range(4 * qg, 4 * qg + 4):
                    for eg in range(2):
                        units.append(u_wo(tt, eg))
                return units

            # eager projections for tg0
            for u in make_proj_units(0):
                u()

            # ---------------- attention, qg-outer ---------------------------
            for qg in range(NTG):
                flushq(proj_fill)           # tg == qg fully emitted
                if qg + 1 < NTG:
                    proj_fill.extend(make_proj_units(qg + 1))
                nkc = 4 * qg + 4
                qs0 = qg * TG

                for pair in range(2):
                    h0, h1 = 2 * pair, 2 * pair + 1
                    ypss = {}
                    combs = {}
                    yr65s = {}
                    for h in (h0, h1):
                        ypss[h] = psp.tile([DH + 1, TG], F32, name=f"py{h}",
                                           tag="y", bufs=2)
                        combs[h] = pool.tile([128, TG], BF16, name=f"cb{h}",
                                             tag="comb", bufs=4)
                    for kc in range(nkc):
                        j = kc - 4 * qg
                        c0 = 128 * j if j > 0 else 0
                        N = TG - c0
                        ks = slice(kc * KC, (kc + 1) * KC)
                        qsl = slice(qs0 + c0, qs0 + TG)
                        sps = {}
                        for h in (h0, h1):
                            sps[h] = psp.tile([128, TG], F32, name=f"ps{h}",
                                              tag="s", bufs=3)
                        # scores: K=64 c-part (row-tiled pair) ...
                        for h in (h0, h1):
                            hp = h % 2
                            nc.tensor.matmul(
                                sps[h][:, 0:N],
                                kc2[pair][hp * 64:(hp + 1) * 64, ks],
                                qc2[pair][hp * 64:(hp + 1) * 64, qsl],
                                start=True, stop=False)
                        # ... + K=32 rope part (row-tiled pair)
                        for h in (h0, h1):
                            tp = (h * DR, 0) if h == 3 else None
                            nc.tensor.matmul(
                                sps[h][:, 0:N],
                                krT[h * DR:(h + 1) * DR, ks],
                                qr_t[h * DR:(h + 1) * DR, qsl],
                                start=False, stop=True, tile_position=tp)
                        for h in (h0, h1):
                            pt = pool.tile([128, TG], BF16, name=f"pt{h}",
                                           tag="pt", bufs=6)
                            nc.scalar.activation(
                                pt[:, 0:N], sps[h][:, 0:N], AF.Exp, scale=SCALE)
                            if j >= 0:
                                nc.gpsimd.tensor_mul(
                                    pt[:, 0:128], pt[:, 0:128], tri_sb)
                            if kc == 0:
                                nc.vector.tensor_copy(combs[h], pt[:, 0:TG])
                            else:
                                nc.vector.tensor_max(
                                    combs[h][:, c0:TG], combs[h][:, c0:TG],
                                    pt[:, 0:N])
                            nc.tensor.matmul(
                                ypss[h][:, c0:TG], v_sb[:, kc, h, 0:DH + 1],
                                pt[:, 0:N],
                                start=(kc == 0), stop=(kc == nkc - 1))
                        drip(proj_fill, 1)
                        drip(wo_fill, 1)

                    # per-head epilogue: yraw+dsum copy, comb folds, transpose
                    for h in (h0, h1):
                        yr65 = pool.tile([DH + 1, TG], BF16, name=f"yr{h}",
                                         tag="yr", bufs=4)
                        nc.vector.tensor_copy(yr65, ypss[h])
                        yr65s[h] = yr65
                        t64 = pool.tile([64, TG], BF16, name="t64", tag="t64",
                                        bufs=2)
                        nc.sync.dma_start(out=t64, in_=combs[h][64:128, :])
                        f64 = pool.tile([64, TG], BF16, name="f64", tag="f64",
                                        bufs=2)
                        nc.vector.tensor_max(f64, combs[h][0:64, :], t64)
                        t32 = pool.tile([32, TG], BF16, name="t32", tag="t32",
                                        bufs=2)
                        nc.sync.dma_start(out=t32, in_=f64[32:64, :])
                        f32b = pool.tile([32, TG], BF16, name="f32b", tag="f32",
                                         bufs=2)
                        nc.vector.tensor_max(f32b, f64[0:32, :], t32)
                        t16 = pool.tile([16, TG], BF16, name="t16", tag="t16",
                                        bufs=2)
                        nc.sync.dma_start(out=t16, in_=f32b[16:32, :])
                        f16t = pool.tile([16, TG], BF16, name="f16t", tag="f16",
                                         bufs=2)
                        nc.vector.tensor_max(f16t, f32b[0:16, :], t16)
                        # restack [16,512] -> rows 0..63 ; dsum -> rows 64..67
                        for a in range(4):
                            nc.sync.dma_start(
                                out=tp_in[16 * a:16 * a + 16, :],
                                in_=f16t[:, 128 * a:128 * (a + 1)])
                        nc.sync.dma_start(
                            out=tp_in[64:68, :],
                            in_=yr65[DH:DH + 1, :].rearrange(
                                "o (a j) -> o a j", a=4))
                        ct = psp.tile([128, 128], BF16, name="ct", tag="pj",
                                      bufs=3)
                        nc.tensor.transpose(ct, tp_in, eyeb_sb)
                        red = pool.tile([128, 4], F32, name="red", tag="red",
                                        bufs=2)
                        nc.vector.reduce_max(
                            red, ct[:, 0:64].rearrange("p (a i) -> p a i", i=16),
                            axis=AX.X)
                        # d = dsum + C*emax   (both in transposed layout)
                        nc.vector.scalar_tensor_tensor(
                            out=dAll[:, qg, h, :], in0=red,
                            scalar=c_bcast[:, h:h + 1], in1=ct[:, 64:68],
                            op0=ALU.mult, op1=ALU.add)
                        drip(proj_fill, 1)
                        drip(wo_fill, 1)

                    # normalize this pair once r is available (deferred to
                    # after both pairs: see below).  Stash yr65s on the pair.
                    if pair == 0:
                        yr65s_a = yr65s

                # ---- qg epilogue: 1/d, r rows, broadcast, normalize, Wo ----
                nc.vector.reciprocal(rAll[:, qg, :, :], dAll[:, qg, :, :])
                rT_ps = psp.tile([16, 128], F32, name="rT_ps", tag="pj", bufs=3)
                nc.tensor.transpose(
                    rT_ps, rAll[:, qg, :, :].rearrange("p h a -> p (h a)"),
                    eyef_sb)
                rT_sb = pool.tile([16, 128], BF16, name="rT_sb", tag="rT",
                                  bufs=2)
                nc.vector.tensor_copy(rT_sb, rT_ps)
                all_yr = {**yr65s_a, **yr65s}
                for h in range(HL):
                    r_row = pool.tile([1, TG], BF16, name="r_row", tag="rrow",
                                      bufs=2)
                    nc.sync.dma_start(
                        out=r_row.rearrange("o (a j) -> o a j", a=4),
                        in_=rT_sb[4 * h:4 * h + 4, :])
                    rbc = psp.tile([64, TG], F32, name="rbc", tag="pj", bufs=3)
                    nc.tensor.matmul(
                        rbc, ones1[:, 0:64], r_row, start=True, stop=True)
                    if h % 2 == 0:
                        nc.vector.tensor_mul(
                            yns[h // 2][0:64, qs0:qs0 + TG],
                            all_yr[h][0:DH, :], rbc)
                    else:
                        yt = pool.tile([64, TG], BF16, name="yt", tag="yt",
                                       bufs=2)
                        nc.vector.tensor_mul(yt, all_yr[h][0:DH, :], rbc)
                        nc.sync.dma_start(
                            out=yns[h // 2][64:128, qs0:qs0 + TG], in_=yt)
                wo_fill.extend(make_wo_units(qg))
                if qg == NTG - 1:
                    flushq(proj_fill)
                    flushq(wo_fill)

    return nc


def _rope_tables():
    half = DR // 2
    inv = 1.0 / (10000.0 ** (np.arange(half, dtype=np.float64) / half))
    ang = np.arange(T, dtype=np.float64)[:, None] * inv[None, :]  # (T, half)
    cos = np.cos(ang).T  # (half, T)
    sin = np.sin(ang).T
    cosk = np.concatenate([cos, cos], axis=0)                 # (32, T)
    sink = np.concatenate([-sin, sin], axis=0)
    cosq = np.tile(cosk, (HL, 1)).astype(np.float32)          # (128, T)
    sinq = np.tile(sink, (HL, 1)).astype(np.float32)
    return cosq, sinq


def kernel(x, Wq, Wqr, Wkr, Wkvd, Wku, Wvu, Wo, lobo_log):
    import ml_dtypes
    bf16 = ml_dtypes.bfloat16

    x = np.asarray(x, dtype=np.float32)
    Wq = np.asarray(Wq, dtype=np.float32)
    Wqr = np.asarray(Wqr, dtype=np.float32)
    Wkr = np.asarray(Wkr, dtype=np.float32)
    Wkvd = np.asarray(Wkvd, dtype=np.float32)
    Wku = np.asarray(Wku, dtype=np.float32)
    Wvu = np.asarray(Wvu, dtype=np.float32)
    Wo = np.asarray(Wo, dtype=np.float32)
    lobo_log = np.asarray(lobo_log, dtype=np.float32)

    if "nc" not in _CACHE:
        _CACHE["nc"] = _build_program()
    nc = _CACHE["nc"]

    cosq, sinq = _rope_tables()
    tri = (np.arange(128)[:, None] <= np.arange(128)[None, :])
    tri = tri.astype(np.float32)
    eye = np.eye(128, dtype=np.float32)

    def b16(a):
        return np.ascontiguousarray(a.astype(bf16))

    in_maps = []
    for core in range(8):
        b, g = core // NG, core % NG
        hs = slice(g * HL * DH, (g + 1) * HL * DH)
        rs = slice(g * HL * DR, (g + 1) * HL * DR)
        in_maps.append({
            "xT": b16(x[b].T),
            "wq": b16(Wq[:, hs]),
            "wqr": b16(Wqr[:, rs]),
            "wkr": b16(Wkr),
            "wkvd": b16(Wkvd),
            "wku": b16(Wku[:, hs]),
            "wvu": b16(Wvu[:, hs]),
            "wo": b16(Wo[hs, :]),
            "cosq": b16(cosq), "sinq": b16(sinq),
            "tri": b16(tri), "eyeb": b16(eye),
            "eyef": np.ascontiguousarray(eye),
            "lobo": np.ascontiguousarray(
                lobo_log[g * HL:(g + 1) * HL].reshape(1, HL)),
        })

    trace = bool(os.environ.get("BASS_TRACE_KERNEL"))
    bkr = run_bass_kernel_spmd(
        nc, in_maps, core_ids=list(range(8)), trace=trace)
    if trace:
        print(f"HW exec time: {bkr.exec_time_ns} ns")
        if bkr.instructions_and_trace is not None:
            print("trace:", bkr.instructions_and_trace[1])
        _CACHE["last_result"] = bkr
    res = bkr.results
    out = np.zeros((B, T, E), dtype=np.float32)
    for core in range(8):
        out[core // NG] += res[core]["out"].astype(np.float32)
    return out
